# revision 12
# baseline (speedup 1.0000x reference)
"""Trainium2 Bass kernel for nn_ConditionMask (pooling / memory-bound).

Math: the module's four Linears act on K-independent features, so the whole
network collapses to, per (b, l):

    val[b,l] = sum_c T[b,l,c]*u_time[c]           (time_embed part)
             + sum_k F[b,k,l]*u_feat[k]           (feature_embed part)
             + u0*mean[b,l] + u1*std[b,l] + u2*resid[b,l]   (local stats)
             + C
    out[b, c, k, l] = val[b, l]   broadcast over (2*CHANNELS, K)

where u_time = W_time.T @ W_final[0,:128], u_feat = W_feat.T @ W_final[0,128:144],
u_stat = W_stat.T @ W_final[0,144:272], C = bias terms folded.

Sharding: pure data parallel, B=16 split 2 batches per core over 8 cores.
Output dominates HBM traffic: 64 MiB total writes (8 MiB/core).

Engine layout notes: per-b stat rows live at partitions 0 and 32 (not 0/1)
because PE outputs and engine APs must start at 32-aligned partitions.
"""

import numpy as np

B, K, L = 16, 64, 512
CHANNELS = 16
EPS = 1e-8
N_CORES = 8
BPC = B // N_CORES  # 2 batches per core
OUT_ROWS = BPC * 2 * CHANNELS * K  # 4096 rows of length L per core
SW = 33  # stat tile partition count: b0 rows at 0, b1 rows at 32

_cached_nc = None


def _build_program():
    from contextlib import ExitStack

    import concourse.bacc as bacc
    import concourse.tile as tile
    from concourse import mybir
    from concourse.masks import make_identity

    f32 = mybir.dt.float32
    Alu = mybir.AluOpType

    nc = bacc.Bacc(
        "TRN2",
        target_bir_lowering=False,
        debug=False,
        num_devices=N_CORES,
    )

    obs_d = nc.dram_tensor("obs", [128, 512], f32, kind="ExternalInput").ap()
    tgt_d = nc.dram_tensor("tgt", [128, 512], f32, kind="ExternalInput").ap()
    feat_d = nc.dram_tensor("feat", [128, 512], f32, kind="ExternalInput").ap()
    temb_d = nc.dram_tensor("temb", [1024, 128], f32, kind="ExternalInput").ap()
    # cpack cols: 0..32 block-ones (b0 at col0, b1 at col32), 33..65 block-u_feat,
    # 66 u_time
    cpack_d = nc.dram_tensor("cpack", [128, 67], f32, kind="ExternalInput").ap()
    # cvec cols (replicated on all SW partitions): u0, u1, u2/64, C
    cvec_d = nc.dram_tensor("cvec", [SW, 4], f32, kind="ExternalInput").ap()
    out_d = nc.dram_tensor("out", [OUT_ROWS, L], f32, kind="ExternalOutput").ap()

    with tile.TileContext(nc) as tc, ExitStack() as ctx:
        consts = ctx.enter_context(tc.tile_pool(name="consts", bufs=1))
        data = ctx.enter_context(tc.tile_pool(name="data", bufs=1))
        work = ctx.enter_context(tc.tile_pool(name="work", bufs=1))
        ps_s_pool = ctx.enter_context(tc.tile_pool(name="psS", bufs=1, space="PSUM"))
        ps_b_pool = ctx.enter_context(tc.tile_pool(name="psB", bufs=2, space="PSUM"))

        cpack = consts.tile([128, 67], f32)
        nc.sync.dma_start(out=cpack, in_=cpack_d)
        cvec = consts.tile([SW, 4], f32)
        nc.sync.dma_start(out=cvec, in_=cvec_d)
        ident = consts.tile([128, 128], f32)
        make_identity(nc, ident)
        ones1 = consts.tile([1, 128], f32)
        nc.vector.memset(ones1, 1.0)

        blk = cpack[:, 0:SW]
        wblk = cpack[:, SW : 2 * SW]
        ut = cpack[:, 2 * SW : 2 * SW + 1]
        u0 = cvec[:, 0:1]
        u1 = cvec[:, 1:2]
        u2 = cvec[:, 2:3]

        obs = data.tile([128, 512], f32)
        nc.sync.dma_start(out=obs, in_=obs_d)
        tgt = data.tile([128, 512], f32)
        nc.sync.dma_start(out=tgt, in_=tgt_d)
        feat = data.tile([128, 512], f32)
        nc.sync.dma_start(out=feat, in_=feat_d)
        temb = data.tile([128, 8, 128], f32)
        nc.sync.dma_start(out=temb, in_=temb_d.rearrange("(n p) d -> p n d", p=128))

        masked = work.tile([128, 512], f32)
        nc.vector.tensor_mul(masked, obs, tgt)
        masked2 = work.tile([128, 512], f32)
        nc.scalar.square(masked2, masked)

        # Per-b reductions over K via block matmuls -> [SW, 512] PSUM tiles
        # (row 0 = b0, row 32 = b1; other rows are zeros).
        ps_d = ps_s_pool.tile([SW, 512], f32)
        nc.tensor.matmul(ps_d, blk, tgt)
        ps_s = ps_s_pool.tile([SW, 512], f32)
        nc.tensor.matmul(ps_s, blk, masked)
        ps_ss = ps_s_pool.tile([SW, 512], f32)
        nc.tensor.matmul(ps_ss, blk, masked2)
        ps_om = ps_s_pool.tile([SW, 512], f32)
        nc.tensor.matmul(ps_om, blk, obs)
        ps_vf = ps_s_pool.tile([SW, 512], f32)
        nc.tensor.matmul(ps_vf, wblk, feat)

        # Time part: transpose [l, c] chunks to [c, l], then u_time reduction.
        # vt rows live at partitions 0 / 32 of one PSUM bank.
        ps_vt = ps_s_pool.tile([SW, 512], f32)
        for b in range(BPC):
            tt_ps = ps_b_pool.tile([128, 512], f32, tag="big")
            for i in range(4):
                nc.tensor.transpose(
                    tt_ps[:, i * 128 : (i + 1) * 128], temb[:, b * 4 + i, :], ident
                )
            tt_sb = work.tile([128, 512], f32, tag=f"ttsb{b}")
            nc.scalar.copy(tt_sb, tt_ps)
            nc.tensor.matmul(
                ps_vt[32 * b : 32 * b + 1, :], ut, tt_sb, skip_group_check=True
            )

        # Stats chain on [SW, 512] tiles (free-dim bound, so same cost as 2 rows).
        dn = work.tile([SW, 512], f32)
        nc.vector.tensor_scalar(out=dn, in0=ps_d, scalar1=EPS, scalar2=None, op0=Alu.add)
        inv = work.tile([SW, 512], f32)
        nc.vector.reciprocal(inv, dn)
        mean = work.tile([SW, 512], f32)
        nc.vector.tensor_mul(mean, ps_s, inv)
        # var = max(ss - 2*mean*(s - 32*mean), 0) * inv ; std = sqrt(var)
        t1 = work.tile([SW, 512], f32)
        nc.vector.scalar_tensor_tensor(
            out=t1, in0=mean, scalar=-32.0, in1=ps_s, op0=Alu.mult, op1=Alu.add
        )
        t2 = work.tile([SW, 512], f32)
        nc.vector.tensor_mul(t2, mean, t1)
        t3 = work.tile([SW, 512], f32)
        nc.vector.scalar_tensor_tensor(
            out=t3, in0=t2, scalar=-2.0, in1=ps_ss, op0=Alu.mult, op1=Alu.add
        )
        var = work.tile([SW, 512], f32)
        nc.vector.scalar_tensor_tensor(
            out=var, in0=t3, scalar=0.0, in1=inv, op0=Alu.max, op1=Alu.mult
        )
        std = work.tile([SW, 512], f32)
        nc.scalar.sqrt(std, var)

        # Residual-mean part (on raw column sums; 1/64 folded into u2).
        om = work.tile([SW, 512], f32)
        nc.scalar.copy(om, ps_om)
        p1 = work.tile([SW, 508], f32)
        nc.gpsimd.tensor_add(p1, om[:, 0:508], om[:, 1:509])
        p2 = work.tile([SW, 508], f32)
        nc.gpsimd.tensor_add(p2, om[:, 2:510], om[:, 3:511])
        p3 = work.tile([SW, 508], f32)
        nc.gpsimd.tensor_add(p3, p1, p2)
        s5 = work.tile([SW, 508], f32)
        nc.gpsimd.tensor_add(s5, p3, om[:, 4:512])
        resid = work.tile([SW, 512], f32)
        nc.gpsimd.memset(resid, 0.0)
        nc.vector.scalar_tensor_tensor(
            out=resid[:, 2:510],
            in0=s5,
            scalar=-0.2,
            in1=om[:, 2:510],
            op0=Alu.mult,
            op1=Alu.add,
        )

        # val = u0*mean + u1*std + u2*resid + vf + vt + C
        a1 = work.tile([SW, 512], f32)
        nc.vector.scalar_tensor_tensor(
            out=a1, in0=mean, scalar=u0, in1=ps_vf, op0=Alu.mult, op1=Alu.add
        )
        a2 = work.tile([SW, 512], f32)
        nc.vector.scalar_tensor_tensor(
            out=a2, in0=std, scalar=u1, in1=a1, op0=Alu.mult, op1=Alu.add
        )
        a3 = work.tile([SW, 512], f32)
        nc.vector.scalar_tensor_tensor(
            out=a3, in0=resid, scalar=u2, in1=a2, op0=Alu.mult, op1=Alu.add
        )
        val = []
        for b in range(BPC):
            val_b = work.tile([1, 512], f32, tag=f"val{b}")
            nc.vector.scalar_tensor_tensor(
                out=val_b,
                in0=ps_vt[32 * b : 32 * b + 1, :],
                scalar=cvec[32 * b : 32 * b + 1, 3:4],
                in1=a3[32 * b : 32 * b + 1, :],
                op0=Alu.add,
                op1=Alu.add,
            )
            val.append(val_b)

        # Broadcast val row across 128 partitions and stream out.
        for b in range(BPC):
            bc_ps = ps_b_pool.tile([128, 512], f32, tag="big")
            nc.tensor.matmul(bc_ps, ones1, val[b])
            bc_sb = work.tile([128, 512], f32, tag=f"bc{b}")
            if b == 0:
                nc.vector.tensor_copy(bc_sb, bc_ps)
            else:
                nc.scalar.copy(bc_sb, bc_ps)
            for j in range(16):
                r0 = (b * 16 + j) * 128
                eng = nc.sync if j % 2 == 0 else nc.scalar
                eng.dma_start(out=out_d[r0 : r0 + 128, :], in_=bc_sb)

    nc.compile()
    return nc


def _host_consts(W_stat, b_stat, W_time, b_time, W_feat, b_feat, W_final, b_final):
    wt = W_final[0, :128].astype(np.float64)
    wf = W_final[0, 128:144].astype(np.float64)
    ws = W_final[0, 144:272].astype(np.float64)
    u_time = W_time.astype(np.float64).T @ wt  # [128]
    u_feat = W_feat.astype(np.float64).T @ wf  # [64]
    u_stat = W_stat.astype(np.float64).T @ ws  # [3]
    cconst = (
        b_time.astype(np.float64) @ wt
        + b_feat.astype(np.float64) @ wf
        + b_stat.astype(np.float64) @ ws
        + float(b_final[0])
    )
    cpack = np.zeros((128, 67), np.float32)
    cpack[0:64, 0] = 1.0
    cpack[64:128, 32] = 1.0
    cpack[0:64, SW] = u_feat.astype(np.float32)
    cpack[64:128, SW + 32] = u_feat.astype(np.float32)
    cpack[:, 2 * SW] = u_time.astype(np.float32)
    cvec = np.zeros((SW, 4), np.float32)
    cvec[:, 0] = u_stat[0]
    cvec[:, 1] = u_stat[1]
    cvec[:, 2] = u_stat[2] / K
    cvec[:, 3] = cconst
    return cpack, cvec


def kernel(
    observed_data,
    x_mask,
    cond_mask,
    target_mask,
    time_embed,
    feature_embed,
    W_stat,
    b_stat,
    W_time,
    b_time,
    W_feat,
    b_feat,
    W_final,
    b_final,
):
    global _cached_nc
    from concourse.bass_utils import run_bass_kernel_spmd

    observed_data = np.ascontiguousarray(np.asarray(observed_data, np.float32))
    target_mask = np.ascontiguousarray(np.asarray(target_mask, np.float32))
    feature_embed = np.ascontiguousarray(np.asarray(feature_embed, np.float32))
    time_embed = np.ascontiguousarray(np.asarray(time_embed, np.float32))

    cpack, cvec = _host_consts(
        np.asarray(W_stat), np.asarray(b_stat), np.asarray(W_time),
        np.asarray(b_time), np.asarray(W_feat), np.asarray(b_feat),
        np.asarray(W_final), np.asarray(b_final),
    )

    if _cached_nc is None:
        _cached_nc = _build_program()
    nc = _cached_nc

    in_maps = []
    for c in range(N_CORES):
        bsl = slice(c * BPC, (c + 1) * BPC)
        in_maps.append(
            {
                "obs": observed_data[bsl].reshape(BPC * K, L),
                "tgt": target_mask[bsl].reshape(BPC * K, L),
                "feat": feature_embed[bsl].reshape(BPC * K, L),
                "temb": time_embed[bsl].reshape(BPC * L, 128),
                "cpack": cpack,
                "cvec": cvec,
            }
        )

    res = run_bass_kernel_spmd(nc, in_maps, core_ids=list(range(N_CORES)))
    outs = [
        r["out"].reshape(BPC, 2 * CHANNELS, K, L) for r in res.results
    ]
    return np.concatenate(outs, axis=0)


# revision 14
# speedup vs baseline: 1.0427x; 1.0427x over previous
"""Trainium2 Bass kernel for nn_ConditionMask (pooling / memory-bound).

Math: the module's four Linears act on K-independent features, so the whole
network collapses to, per (b, l):

    val[b,l] = sum_c T[b,l,c]*u_time[c]           (time_embed part)
             + sum_k F[b,k,l]*u_feat[k]           (feature_embed part)
             + u0*mean[b,l] + u1*std[b,l] + u2*resid[b,l]   (local stats)
             + C
    out[b, c, k, l] = val[b, l]   broadcast over (2*CHANNELS, K)

where u_time = W_time.T @ W_final[0,:128], u_feat = W_feat.T @ W_final[0,128:144],
u_stat = W_stat.T @ W_final[0,144:272], C = bias terms folded.

Sharding: pure data parallel, B=16 split 2 batches per core over 8 cores.
Output dominates HBM traffic: 64 MiB total writes (8 MiB/core).

Layout/precision notes:
- Per-b stat rows live at partitions 0 and 32 (PE outputs and engine APs must
  start at 32-aligned partitions).
- K-reduction matmuls run in bf16 (masks are 0/1 so the denominator count is
  exact; measured end-to-end max rel err 6e-4). The time_embed path and the
  final broadcast stay fp32.
- Residual: r = obs - 0.2*rolling5(obs) computed per-k in bf16, then one
  block matmul sums over K; PSUM is pre-zeroed so the l-edges stay 0.
"""

import numpy as np

B, K, L = 16, 64, 512
CHANNELS = 16
EPS = 1e-8
N_CORES = 8
BPC = B // N_CORES  # 2 batches per core
OUT_ROWS = BPC * 2 * CHANNELS * K  # 4096 rows of length L per core
SW = 33  # stat tile partition count: b0 rows at 0, b1 rows at 32

_cached_nc = None


def _build_program():
    from contextlib import ExitStack

    import concourse.bacc as bacc
    import concourse.tile as tile
    from concourse import mybir
    from concourse.masks import make_identity

    f32 = mybir.dt.float32
    bf16 = mybir.dt.bfloat16
    Alu = mybir.AluOpType

    nc = bacc.Bacc(
        "TRN2",
        target_bir_lowering=False,
        debug=False,
        num_devices=N_CORES,
    )

    obs_d = nc.dram_tensor("obs", [128, 512], f32, kind="ExternalInput").ap()
    tgt_d = nc.dram_tensor("tgt", [128, 512], f32, kind="ExternalInput").ap()
    feat_d = nc.dram_tensor("feat", [128, 512], f32, kind="ExternalInput").ap()
    temb_d = nc.dram_tensor("temb", [1024, 128], f32, kind="ExternalInput").ap()
    # cpack cols: 0..32 block-ones (b0 at col0, b1 at col32), 33..65 block-u_feat
    cpack_d = nc.dram_tensor("cpack", [128, 66], bf16, kind="ExternalInput").ap()
    ut_d = nc.dram_tensor("ut", [128, 1], f32, kind="ExternalInput").ap()
    # cvec cols (replicated on all SW partitions): u0, u1, u2/64, C
    cvec_d = nc.dram_tensor("cvec", [SW, 4], f32, kind="ExternalInput").ap()
    out_d = nc.dram_tensor("out", [OUT_ROWS, L], f32, kind="ExternalOutput").ap()

    with tile.TileContext(nc) as tc, ExitStack() as ctx:
        consts = ctx.enter_context(tc.tile_pool(name="consts", bufs=1))
        data = ctx.enter_context(tc.tile_pool(name="data", bufs=1))
        work = ctx.enter_context(tc.tile_pool(name="work", bufs=1))
        ps_s_pool = ctx.enter_context(tc.tile_pool(name="psS", bufs=1, space="PSUM"))
        ps_b_pool = ctx.enter_context(tc.tile_pool(name="psB", bufs=2, space="PSUM"))

        # --- input loads -------------------------------------------------
        # bf16 operands arrive via SWDGE cast-during-DMA (gpsimd ring);
        # fp32 operands via HWDGE (sync ring). Order = need order.
        tgt_bf = data.tile([128, 512], bf16)
        nc.gpsimd.dma_start(out=tgt_bf, in_=tgt_d)
        obs_bf = data.tile([128, 512], bf16)
        nc.gpsimd.dma_start(out=obs_bf, in_=obs_d)
        feat_bf = data.tile([128, 512], bf16)
        nc.gpsimd.dma_start(out=feat_bf, in_=feat_d)

        cpack = consts.tile([128, 66], bf16)
        nc.sync.dma_start(out=cpack, in_=cpack_d)
        cvec = consts.tile([SW, 4], f32)
        nc.sync.dma_start(out=cvec, in_=cvec_d)
        ut = consts.tile([128, 1], f32)
        nc.sync.dma_start(out=ut, in_=ut_d)
        temb = data.tile([128, 8, 128], f32)
        nc.sync.dma_start(out=temb, in_=temb_d.rearrange("(n p) d -> p n d", p=128))

        ident = consts.tile([128, 128], f32)
        make_identity(nc, ident)
        ones1 = consts.tile([1, 128], f32)
        nc.vector.memset(ones1, 1.0)

        blk = cpack[:, 0:SW]
        wblk = cpack[:, SW : 2 * SW]
        u0 = cvec[:, 0:1]
        u1 = cvec[:, 1:2]
        u2 = cvec[:, 2:3]

        # --- PSUM banks (6 stat + 2 big = 8) -----------------------------
        ps_d = ps_s_pool.tile([SW, 512], f32)
        ps_s = ps_s_pool.tile([SW, 512], f32)
        ps_ss = ps_s_pool.tile([SW, 512], f32)
        ps_vf = ps_s_pool.tile([SW, 512], f32)
        ps_r = ps_s_pool.tile([SW, 512], f32)
        ps_vt = ps_s_pool.tile([SW, 512], f32)

        # Prefill: eps into the denominator bank, zeros into the residual
        # bank (so the matmuls below accumulate on top and l-edges stay 0).
        nc.vector.memset(ps_d, EPS)
        nc.vector.memset(ps_r, 0.0)

        # --- elementwise prep (bf16, cheap) ------------------------------
        masked = work.tile([128, 512], bf16)
        nc.vector.tensor_mul(masked, obs_bf, tgt_bf)
        masked2 = work.tile([128, 512], bf16)
        nc.vector.tensor_mul(masked2, masked, masked)

        # rolling-window residual per k (gpsimd; independent of DVE chain)
        q1 = work.tile([128, 508], bf16)
        nc.gpsimd.tensor_add(q1, obs_bf[:, 0:508], obs_bf[:, 1:509])
        q2 = work.tile([128, 508], bf16)
        nc.gpsimd.tensor_add(q2, obs_bf[:, 2:510], obs_bf[:, 3:511])
        q3 = work.tile([128, 508], bf16)
        nc.gpsimd.tensor_add(q3, q1, q2)
        q4 = work.tile([128, 508], bf16)
        nc.gpsimd.tensor_add(q4, q3, obs_bf[:, 4:512])
        rfull = work.tile([128, 508], bf16)
        nc.vector.scalar_tensor_tensor(
            out=rfull, in0=q4, scalar=-0.2, in1=obs_bf[:, 2:510],
            op0=Alu.mult, op1=Alu.add,
        )

        # --- K reductions on PE (bf16 single-pass) -----------------------
        nc.tensor.matmul(ps_d, blk, tgt_bf, start=False, stop=True,
                         skip_group_check=True)
        nc.tensor.matmul(ps_s, blk, masked)
        nc.tensor.matmul(ps_ss, blk, masked2)
        nc.tensor.matmul(ps_vf, wblk, feat_bf)
        nc.tensor.matmul(ps_r[:, 2:510], blk, rfull, start=False, stop=True,
                         skip_group_check=True)

        # --- time path (fp32): transpose chunks, u_time reduction --------
        for b in range(BPC):
            tt_ps = ps_b_pool.tile([128, 512], f32, tag="big")
            for i in range(4):
                nc.tensor.transpose(
                    tt_ps[:, i * 128 : (i + 1) * 128], temb[:, b * 4 + i, :], ident
                )
            tt_sb = work.tile([128, 512], f32, tag=f"ttsb{b}")
            nc.scalar.copy(tt_sb, tt_ps)
            nc.tensor.matmul(
                ps_vt[32 * b : 32 * b + 1, :], ut, tt_sb, skip_group_check=True
            )

        # --- stats chain (fp32, [SW,512] rows) ---------------------------
        inv = work.tile([SW, 512], f32)
        scr = work.tile([SW, 512], f32)
        nc.vector.reciprocal_approx_accurate(out=inv, in_=ps_d, scratch=scr)
        mean = work.tile([SW, 512], f32)
        nc.vector.tensor_mul(mean, ps_s, inv)
        # var = max(ss - 2*mean*(s - 32*mean), 0) * inv ; std = sqrt(var)
        t1 = work.tile([SW, 512], f32)
        nc.vector.scalar_tensor_tensor(
            out=t1, in0=mean, scalar=-32.0, in1=ps_s, op0=Alu.mult, op1=Alu.add
        )
        t2 = work.tile([SW, 512], f32)
        nc.vector.tensor_mul(t2, mean, t1)
        t3 = work.tile([SW, 512], f32)
        nc.vector.scalar_tensor_tensor(
            out=t3, in0=t2, scalar=-2.0, in1=ps_ss, op0=Alu.mult, op1=Alu.add
        )
        var = work.tile([SW, 512], f32)
        nc.vector.scalar_tensor_tensor(
            out=var, in0=t3, scalar=0.0, in1=inv, op0=Alu.max, op1=Alu.mult
        )
        std = work.tile([SW, 512], f32)
        nc.scalar.sqrt(std, var)

        # --- val assembly ------------------------------------------------
        a1 = work.tile([SW, 512], f32)
        nc.vector.scalar_tensor_tensor(
            out=a1, in0=mean, scalar=u0, in1=ps_vf, op0=Alu.mult, op1=Alu.add
        )
        a2 = work.tile([SW, 512], f32)
        nc.vector.scalar_tensor_tensor(
            out=a2, in0=std, scalar=u1, in1=a1, op0=Alu.mult, op1=Alu.add
        )
        a3 = work.tile([SW, 512], f32)
        nc.vector.scalar_tensor_tensor(
            out=a3, in0=ps_r, scalar=u2, in1=a2, op0=Alu.mult, op1=Alu.add
        )
        val = []
        for b in range(BPC):
            val_b = work.tile([1, 512], f32, tag=f"val{b}")
            nc.vector.scalar_tensor_tensor(
                out=val_b,
                in0=ps_vt[32 * b : 32 * b + 1, :],
                scalar=cvec[32 * b : 32 * b + 1, 3:4],
                in1=a3[32 * b : 32 * b + 1, :],
                op0=Alu.add,
                op1=Alu.add,
            )
            val.append(val_b)

        # --- broadcast + output stream -----------------------------------
        for b in range(BPC):
            bc_ps = ps_b_pool.tile([128, 512], f32, tag="big")
            nc.tensor.matmul(bc_ps, ones1, val[b])
            bc_sb = work.tile([128, 512], f32, tag=f"bc{b}")
            if b == 0:
                nc.scalar.copy(bc_sb, bc_ps)
            else:
                nc.vector.tensor_copy(bc_sb, bc_ps)
            for j in range(16):
                r0 = (b * 16 + j) * 128
                eng = nc.sync if j % 2 == 0 else nc.scalar
                eng.dma_start(out=out_d[r0 : r0 + 128, :], in_=bc_sb)

    nc.compile()
    return nc


def _host_consts(W_stat, b_stat, W_time, b_time, W_feat, b_feat, W_final, b_final):
    import ml_dtypes

    wt = W_final[0, :128].astype(np.float64)
    wf = W_final[0, 128:144].astype(np.float64)
    ws = W_final[0, 144:272].astype(np.float64)
    u_time = W_time.astype(np.float64).T @ wt  # [128]
    u_feat = W_feat.astype(np.float64).T @ wf  # [64]
    u_stat = W_stat.astype(np.float64).T @ ws  # [3]
    cconst = (
        b_time.astype(np.float64) @ wt
        + b_feat.astype(np.float64) @ wf
        + b_stat.astype(np.float64) @ ws
        + float(b_final[0])
    )
    cpack = np.zeros((128, 66), np.float32)
    cpack[0:64, 0] = 1.0
    cpack[64:128, 32] = 1.0
    cpack[0:64, SW] = u_feat.astype(np.float32)
    cpack[64:128, SW + 32] = u_feat.astype(np.float32)
    cpack = cpack.astype(ml_dtypes.bfloat16)
    ut = u_time.astype(np.float32).reshape(128, 1)
    cvec = np.zeros((SW, 4), np.float32)
    cvec[:, 0] = u_stat[0]
    cvec[:, 1] = u_stat[1]
    cvec[:, 2] = u_stat[2] / K
    cvec[:, 3] = cconst
    return cpack, ut, cvec


def kernel(
    observed_data,
    x_mask,
    cond_mask,
    target_mask,
    time_embed,
    feature_embed,
    W_stat,
    b_stat,
    W_time,
    b_time,
    W_feat,
    b_feat,
    W_final,
    b_final,
):
    global _cached_nc
    from concourse.bass_utils import run_bass_kernel_spmd

    observed_data = np.ascontiguousarray(np.asarray(observed_data, np.float32))
    target_mask = np.ascontiguousarray(np.asarray(target_mask, np.float32))
    feature_embed = np.ascontiguousarray(np.asarray(feature_embed, np.float32))
    time_embed = np.ascontiguousarray(np.asarray(time_embed, np.float32))

    cpack, ut, cvec = _host_consts(
        np.asarray(W_stat), np.asarray(b_stat), np.asarray(W_time),
        np.asarray(b_time), np.asarray(W_feat), np.asarray(b_feat),
        np.asarray(W_final), np.asarray(b_final),
    )

    if _cached_nc is None:
        _cached_nc = _build_program()
    nc = _cached_nc

    in_maps = []
    for c in range(N_CORES):
        bsl = slice(c * BPC, (c + 1) * BPC)
        in_maps.append(
            {
                "obs": observed_data[bsl].reshape(BPC * K, L),
                "tgt": target_mask[bsl].reshape(BPC * K, L),
                "feat": feature_embed[bsl].reshape(BPC * K, L),
                "temb": time_embed[bsl].reshape(BPC * L, 128),
                "cpack": cpack,
                "ut": ut,
                "cvec": cvec,
            }
        )

    res = run_bass_kernel_spmd(nc, in_maps, core_ids=list(range(N_CORES)))
    outs = [
        r["out"].reshape(BPC, 2 * CHANNELS, K, L) for r in res.results
    ]
    return np.concatenate(outs, axis=0)
